# revision 2
# baseline (speedup 1.0000x reference)
"""3D Haar DWT (nn_Patcher) Trainium2 Bass kernel, v16: H-sharded cores.

Sharding (8 cores): core k -> (b = k//2, hh = k%2); input shard
x[b, :, :, 128*hh:128*hh+128, :] -> [3, 32, 128, 256] (12.58 MB); output
shard y[b, :, :, 64*hh:64*hh+64, :] -> [24, 16, 64, 128].

Keeping all 32 frames per core makes partition p = (t16, h8) legal, which
buys the best DMA shape this transform admits under the 3-dim AP limit:
  input  2x 2 MB per channel, 16 KB contiguous HBM runs
  output 1x 4 MB per channel ((t h8) merges), 4 KB runs
i.e. 6+3 mid-kernel transfers vs the T-sharded layout's 12+5 with 8 KB/2 KB
runs.  SBUF: plain channel-iters use TWO 4 MB tiles, each written twice
(A: input then H-output, B: T-output then W-output); both aliases are
"benign" -- the space's next writer waits only on compute, which leads DMA.
Edge channel-iters (c=0 head, c=2 tail) run as 4 row-quarter sub-iters with
separate small tiles (sub-tile aliasing would clobber later quarters'
inputs), single-engine chains and ring-split outputs at the very tail.
"""

import sys

for _p in ("/opt/trn_rl_repo", "/opt/pypackages"):
    if _p not in sys.path:
        sys.path.append(_p)

import numpy as np

_NC_CACHE = {}


def _build(reps=1):
    if reps in _NC_CACHE:
        return _NC_CACHE[reps]

    from concourse import bacc, mybir
    from concourse.tile import TileContext

    fp32 = mybir.dt.float32
    add = mybir.AluOpType.add
    sub = mybir.AluOpType.subtract

    nc = bacc.Bacc(None, target_bir_lowering=False)
    x = nc.dram_tensor("x_shard", [3, 32, 128, 256], fp32, kind="ExternalInput")
    y = nc.dram_tensor("y_shard", [24, 16, 64, 128], fp32, kind="ExternalOutput")

    # y as [c, (t h8), s, (hr w)]: 4 KB runs ((t h8) merges: 8192 == 8*1024)
    yv = y[:].rearrange("(s c) t (h8 hr) w -> c (t h8) s (hr w)", s=8, c=3, hr=8)
    ycq = y[:].rearrange("(s c) t (h8 hr) w -> c (t h8) s hr w", s=8, c=3, hr=8)

    in_cycle = [nc.sync, nc.scalar]
    out_cycle = [nc.scalar, nc.sync]
    state = {"ni": 0, "no": 0}

    V = nc.vector
    P = nc.gpsimd

    def in_ring():
        r = in_cycle[state["ni"] % 2]
        state["ni"] += 1
        return r

    def out_ring():
        r = out_cycle[state["no"] % 2]
        state["no"] += 1
        return r

    def plain_iter(pool, c):
        """Full 4 MB channel-iter with double-aliased tiles."""
        A = pool.tile([128, 8192], fp32)  # input, then H-output
        B = pool.tile([128, 8192], fp32)  # T-output, then W-output
        Av_in = A.rearrange("p (f r w) -> p f r w", f=2, r=16, w=256)
        Av_h = A.rearrange("p (tb hb r w) -> p tb hb r w", tb=2, hb=2, r=8, w=256)
        Av_hw = A.rearrange(
            "p (tb hb r wh wl) -> p tb hb r wh wl", tb=2, hb=2, r=8, wh=128, wl=2
        )
        Bv_t = B.rearrange("p (tb r w) -> p tb r w", tb=2, r=16, w=256)
        Bv_w = B.rearrange(
            "p (tb hb wb r w) -> p tb hb wb r w", tb=2, hb=2, wb=2, r=8, w=128
        )
        for f in range(2):
            src = x[c, f::2].rearrange("t (h8 r) w -> t h8 (r w)", h8=8)
            dst = Av_in[:, f].rearrange("p r w -> p (r w)")
            in_ring().dma_start(out=dst, in_=src)
        V.tensor_tensor(out=Bv_t[:, 0], in0=Av_in[:, 0], in1=Av_in[:, 1], op=add)
        P.tensor_tensor(out=Bv_t[:, 1], in0=Av_in[:, 0], in1=Av_in[:, 1], op=sub)
        V.tensor_tensor(
            out=Av_h[:, :, 0],
            in0=Bv_t[:, :, 0::2],
            in1=Bv_t[:, :, 1::2],
            op=add,
        )
        P.tensor_tensor(
            out=Av_h[:, :, 1],
            in0=Bv_t[:, :, 0::2],
            in1=Bv_t[:, :, 1::2],
            op=sub,
        )
        V.tensor_tensor(
            out=Bv_w[:, :, :, 0],
            in0=Av_hw[:, :, :, :, :, 0],
            in1=Av_hw[:, :, :, :, :, 1],
            op=add,
        )
        P.tensor_tensor(
            out=Bv_w[:, :, :, 1],
            in0=Av_hw[:, :, :, :, :, 0],
            in1=Av_hw[:, :, :, :, :, 1],
            op=sub,
        )
        for shalf in range(2):
            src = Bv_w[:, shalf].rearrange("p b v r w -> p (b v) (r w)")
            out_ring().dma_start(out=yv[c, :, 4 * shalf : 4 * shalf + 4], in_=src)

    def half_tail(pool, c):
        Ah = pool.tile([128, 4096], fp32)
        Bh = pool.tile([128, 4096], fp32)
        t_in = Ah.rearrange("p (f r w) -> p f r w", f=2, r=8, w=256)
        t_t = Bh.rearrange("p (tb r w) -> p tb r w", tb=2, r=8, w=256)
        t_h = Ah.rearrange("p (a b r w) -> p a b r w", a=2, b=2, r=4, w=256)
        t_o = Bh.rearrange(
            "p (a b v r w) -> p a b v r w", a=2, b=2, v=2, r=4, w=128
        )
        for f in range(2):
            src = x[c, f::2].rearrange(
                "t (h8 rh r) w -> rh t h8 (r w)", h8=8, rh=2
            )[0]
            dst = t_in[:, f].rearrange("p r w -> p (r w)")
            in_ring().dma_start(out=dst, in_=src)
        V.tensor_tensor(out=t_t[:, 0], in0=t_in[:, 0], in1=t_in[:, 1], op=add)
        P.tensor_tensor(out=t_t[:, 1], in0=t_in[:, 0], in1=t_in[:, 1], op=sub)
        V.tensor_tensor(
            out=t_h[:, :, 0], in0=t_t[:, :, 0::2], in1=t_t[:, :, 1::2], op=add
        )
        P.tensor_tensor(
            out=t_h[:, :, 1], in0=t_t[:, :, 0::2], in1=t_t[:, :, 1::2], op=sub
        )
        t_hv = t_h.rearrange("p a b r (wh wl) -> p a b r wh wl", wl=2)
        V.tensor_tensor(
            out=t_o[:, :, :, 0],
            in0=t_hv[:, :, :, :, :, 0], in1=t_hv[:, :, :, :, :, 1], op=add,
        )
        P.tensor_tensor(
            out=t_o[:, :, :, 1],
            in0=t_hv[:, :, :, :, :, 0], in1=t_hv[:, :, :, :, :, 1], op=sub,
        )
        src = t_o.rearrange("p a b v r w -> p (a b v) (r w)")
        dst = ycq[c, :, :, 0:4].rearrange("p s hr w -> p s (hr w)")
        out_ring().dma_start(out=dst, in_=src)

    def sub_iter(pool, c, rq, eng=None, split_wb=False, it=0):
        """Row-quarter (1 MB) sub-iter with its own (non-aliased) tiles.
        eng=None: V/P split chain; else single-engine chain."""
        Aq = pool.tile([128, 2048], fp32)
        Bq = pool.tile([128, 2048], fp32)
        t_in = Aq.rearrange("p (f r w) -> p f r w", f=2, r=4, w=256)
        t_t = Bq.rearrange("p (tb r w) -> p tb r w", tb=2, r=4, w=256)
        t_h = Aq.rearrange("p (a b r w) -> p a b r w", a=2, b=2, r=2, w=256)
        t_o = Bq.rearrange(
            "p (a b v r w) -> p a b v r w", a=2, b=2, v=2, r=2, w=128
        )
        e0 = eng or V
        e1 = eng or P
        for f in range(2):
            src = x[c, f::2].rearrange(
                "t (h8 rq r) w -> rq t h8 (r w)", h8=8, rq=4
            )[rq]
            dst = t_in[:, f].rearrange("p r w -> p (r w)")
            in_ring().dma_start(out=dst, in_=src)
        e0.tensor_tensor(out=t_t[:, 0], in0=t_in[:, 0], in1=t_in[:, 1], op=add)
        e1.tensor_tensor(out=t_t[:, 1], in0=t_in[:, 0], in1=t_in[:, 1], op=sub)
        e0.tensor_tensor(
            out=t_h[:, :, 0], in0=t_t[:, :, 0::2], in1=t_t[:, :, 1::2], op=add
        )
        e1.tensor_tensor(
            out=t_h[:, :, 1], in0=t_t[:, :, 0::2], in1=t_t[:, :, 1::2], op=sub
        )
        t_hv = t_h.rearrange("p a b r (wh wl) -> p a b r wh wl", wl=2)
        w0e = e0 if (eng or it % 2 == 0) else P
        e_w1 = e1
        w0e.tensor_tensor(
            out=t_o[:, :, :, 0],
            in0=t_hv[:, :, :, :, :, 0],
            in1=t_hv[:, :, :, :, :, 1],
            op=add,
        )
        e_w1.tensor_tensor(
            out=t_o[:, :, :, 1],
            in0=t_hv[:, :, :, :, :, 0],
            in1=t_hv[:, :, :, :, :, 1],
            op=sub,
        )
        hsl = slice(2 * rq, 2 * rq + 2)
        if split_wb:
            for wb, ring in ((0, nc.sync), (1, nc.scalar)):
                src = t_o[:, :, :, wb].rearrange("p a b r w -> p (a b) (r w)")
                dst = ycq[c, :, wb::2, hsl].rearrange("p s hr w -> p s (hr w)")
                ring.dma_start(out=dst, in_=src)
        else:
            src = t_o.rearrange("p a b v r w -> p (a b v) (r w)")
            dst = ycq[c, :, :, hsl].rearrange("p s hr w -> p s (hr w)")
            out_ring().dma_start(out=dst, in_=src)

    with TileContext(nc) as tc:
        with tc.tile_pool(name="pool", bufs=2) as pool, \
                tc.tile_pool(name="edgepool", bufs=2) as edgepool, \
                tc.tile_pool(name="halfpool", bufs=1) as halfpool:
            for rep in range(reps):
                for c in range(3):
                    if c == 2:
                        half_tail(halfpool, c)
                        sub_iter(edgepool, c, 2, eng=V)
                        sub_iter(edgepool, c, 3, eng=P, split_wb=True)
                    else:
                        plain_iter(pool, c)

    nc.finalize()
    _NC_CACHE[reps] = nc
    return nc


def _shard(x):
    x = np.ascontiguousarray(np.asarray(x, dtype=np.float32))
    assert x.shape == (4, 3, 32, 256, 256), x.shape
    in_maps = []
    for k in range(8):
        b, hh = divmod(k, 2)
        in_maps.append(
            {"x_shard": np.ascontiguousarray(x[b, :, :, 128 * hh : 128 * hh + 128])}
        )
    return in_maps


def _run(x, trace=False, **spmd_kwargs):
    from concourse.bass_utils import run_bass_kernel_spmd

    nc = _build()
    in_maps = _shard(x)

    bkr = run_bass_kernel_spmd(nc, in_maps, list(range(8)), trace=trace, **spmd_kwargs)

    out = np.empty((4, 24, 16, 128, 128), dtype=np.float32)
    for k in range(8):
        b, hh = divmod(k, 2)
        out[b, :, :, 64 * hh : 64 * hh + 64] = np.asarray(bkr.results[k]["y_shard"])
    return out, bkr


def kernel(x):
    out, _ = _run(x)
    return out



# revision 3
# speedup vs baseline: 198.5672x; 198.5672x over previous
"""3D Haar DWT Trainium2 kernel, vP2: PE-based T-level + 32KB-run input DMA.

Measured on this part: HBM reads are DMA-descriptor-latency-bound — 16KB
runs (the (t16,h8) layout) stream at 231 GB/s while 32KB runs reach
~457 GB/s (418 GB/s via SWDGE with fp32->bf16 cast); writes run at
~413 GB/s; and input/output DMA do NOT overlap (shared SDMA engines
time-share). The serial DMA floor therefore drops from ~85us to ~61us
per core iff the input is read in 32KB runs.

32KB input runs force partition = (t32, h4block): the T butterfly pairs
then live ACROSS partitions, which only the PE can combine — done here as
one bf16 matmul with a +-1 butterfly matrix (the 3-level Haar + 2*sqrt(2)
rescale has net scale exactly 1, so all subband weights are +-1; bf16
holds them exactly; the SWDGE in-DMA casts inputs to bf16, rel err ~2e-3
<< the 2e-2 gate).

Pipeline per channel c (3 channels, pipelined via tile pools):
  SWDGE in-DMA  x[c] (t h4)(hr w) fp32 -> Xb bf16 [128, 8192] (32KB runs)
  PE x16        Wt.T @ Xb[:, 512k:+512] -> PSUM (fp32, 4 chunks/psum tile)
  ACT x4        PSUM [128,2048] -> G   (T-subbands, partition=(t16,h4,tb))
  V/P           H level: G -> F   (hr pairs, stride-2; H+ on V, H- on P)
  V             W level: F -> G   (w pairs; G free = (hb wb hr wh))
  HWDGE out x2  G[tb::2] -> y     (8KB runs, both rings)
"""

import sys

for _p in ("/opt/trn_rl_repo", "/opt/pypackages"):
    if _p not in sys.path:
        sys.path.append(_p)

import numpy as np

_NC_CACHE = {}


def _butterfly_weights(out_mode="strided"):
    import ml_dtypes

    W = np.zeros((128, 128), np.float32)
    for t_in in range(32):
        for h4 in range(4):
            k = t_in * 4 + h4
            tp = t_in // 2
            for tb in range(2):
                if out_mode == "contig":
                    m = tb * 64 + tp * 4 + h4
                else:  # strided: tb is the partition LSB
                    m = tp * 8 + h4 * 2 + tb
                sign = 1.0 if tb == 0 else (1.0 if t_in % 2 == 0 else -1.0)
                W[k, m] = sign
    return W.astype(ml_dtypes.bfloat16)


def _build(reps=1, loop=False, out_mode="strided", wminus_eng="v", halves=True):
    key = (reps, loop, out_mode, wminus_eng, halves)
    if key in _NC_CACHE:
        return _NC_CACHE[key]

    from concourse import bacc, mybir
    from concourse.tile import TileContext
    from concourse.bass import MemorySpace

    fp32 = mybir.dt.float32
    bf16 = mybir.dt.bfloat16
    add = mybir.AluOpType.add
    sub = mybir.AluOpType.subtract

    nc = bacc.Bacc(None, target_bir_lowering=False)
    x = nc.dram_tensor("x_shard", [3, 32, 128, 256], fp32, kind="ExternalInput")
    y = nc.dram_tensor("y_shard", [24, 16, 64, 128], fp32, kind="ExternalOutput")
    wdram = nc.inline_tensor(_butterfly_weights(out_mode), name="haar_w")

    # input per channel: [(t h4), (hr w)] - 32KB contiguous per partition
    xv = x[:].rearrange("c t (h4 hr) w -> c (t h4) (hr w)", h4=4)
    # output: [c, tb, (t h4), hb, wb, (hr w)] - 8KB contiguous runs
    yv = y[:].rearrange(
        "(tb hb wb c) t (h4 hr) w -> c tb (t h4) hb wb (hr w)",
        tb=2, hb=2, wb=2, c=3, h4=4,
    )
    # half-channel output view: hr16 split as (q2, hr8) - 4KB runs
    yq = y[:].rearrange(
        "(tb hb wb c) t (h4 q hr) w -> c tb q (t h4) hb wb (hr w)",
        tb=2, hb=2, wb=2, c=3, h4=4, q=2,
    )

    V = nc.vector
    P = nc.gpsimd
    A = nc.scalar
    WM = V if wminus_eng == "v" else P

    with TileContext(nc) as tc:
        with tc.tile_pool(name="wpool", bufs=1) as wpool, \
                tc.tile_pool(name="xpool", bufs=3) as xpool, \
                tc.tile_pool(name="fpool", bufs=2) as fpool, \
                tc.tile_pool(name="gpool", bufs=2) as gpool, \
                tc.tile_pool(name="ghpool", bufs=2) as ghpool, \
                tc.tile_pool(name="fhpool", bufs=2) as fhpool, \
                tc.tile_pool(name="g2pool", bufs=3) as g2pool, \
                tc.tile_pool(
                    name="psum", bufs=2, space=MemorySpace.PSUM
                ) as psum:
            Wt = wpool.tile([128, 128], bf16)
            nc.sync.dma_start(out=Wt[:, :], in_=wdram[:])

            def in_stage(c):
                # all in-DMAs are emitted before any compute so Pool's
                # SWDGE descriptor generation isn't queued behind H-/W-
                # of earlier channels (which would serialize the input
                # stream against the compute pipeline)
                Xb = xpool.tile([128, 8192], bf16)  # bf16 input (cast in DMA)
                P.dma_start(out=Xb[:, :], in_=xv[c])
                return Xb

            def half(c, Xb, q):
                """Half-channel pipeline: separate (non-aliased) tiles per
                stage so halves overlap freely across engines."""
                Gh = ghpool.tile([128, 4096], fp32)   # T-out (rows 16q..+16)
                Fh = fhpool.tile([128, 4096], fp32)   # H-out
                G2 = g2pool.tile([128, 4096], fp32)   # W-out
                for j in range(2):
                    acc = psum.tile([128, 2048], fp32)
                    for k in range(4):
                        col = 4096 * q + 2048 * j + 512 * k
                        nc.tensor.matmul(
                            acc[:, 512 * k : 512 * k + 512],
                            Wt[:, :],
                            Xb[:, col : col + 512],
                        )
                    A.copy(out=Gh[:, 2048 * j : 2048 * j + 2048], in_=acc[:, :])
                Gv = Gh.rearrange("p (hr w) -> p hr w", hr=16)
                Fv = Fh.rearrange("p (hb hr w) -> p hb hr w", hb=2, hr=8)
                V.tensor_tensor(
                    out=Fv[:, 0], in0=Gv[:, 0::2], in1=Gv[:, 1::2], op=add
                )
                P.tensor_tensor(
                    out=Fv[:, 1], in0=Gv[:, 0::2], in1=Gv[:, 1::2], op=sub
                )
                Fw = Fh.rearrange(
                    "p (hb hr wh wl) -> p hb hr wh wl", hb=2, hr=8, wl=2
                )
                G2w = G2.rearrange(
                    "p (hb wb hr wh) -> p hb wb hr wh", hb=2, wb=2, hr=8
                )
                V.tensor_tensor(
                    out=G2w[:, :, 0],
                    in0=Fw[:, :, :, :, 0], in1=Fw[:, :, :, :, 1], op=add,
                )
                WM.tensor_tensor(
                    out=G2w[:, :, 1],
                    in0=Fw[:, :, :, :, 0], in1=Fw[:, :, :, :, 1], op=sub,
                )
                if out_mode == "contig":
                    G2s = G2.rearrange("p (hb wb rw) -> p hb wb rw", hb=2, wb=2)
                    nc.sync.dma_start(out=yq[c, 0, q], in_=G2s[0:64])
                    nc.scalar.dma_start(out=yq[c, 1, q], in_=G2s[64:128])
                else:
                    G2t = G2.rearrange("(u two) f -> two u f", two=2)
                    for tb, ring in ((0, nc.sync), (1, nc.scalar)):
                        src_ = G2t[tb].rearrange(
                            "u (hb wb rw) -> u hb wb rw", hb=2, wb=2
                        )
                        ring.dma_start(out=yq[c, tb, q], in_=src_)

            def channel(c, Xb):
                if halves:
                    half(c, Xb, 0)
                    half(c, Xb, 1)
                    return
                F = fpool.tile([128, 8192], fp32)   # H-level output
                G = gpool.tile([128, 8192], fp32)   # T-out, later W-out
                for j in range(4):
                    acc = psum.tile([128, 2048], fp32)
                    for k in range(4):
                        nc.tensor.matmul(
                            acc[:, 512 * k : 512 * k + 512],
                            Wt[:, :],
                            Xb[:, 2048 * j + 512 * k : 2048 * j + 512 * k + 512],
                        )
                    A.copy(out=G[:, 2048 * j : 2048 * j + 2048], in_=acc[:, :])
                # H level: G rows (hr32, w256) -> F (hb2, hr16, w256)
                Gv = G.rearrange("p (hr w) -> p hr w", hr=32)
                Fh = F.rearrange("p (hb hr w) -> p hb hr w", hb=2, hr=16)
                V.tensor_tensor(
                    out=Fh[:, 0], in0=Gv[:, 0::2], in1=Gv[:, 1::2], op=add
                )
                P.tensor_tensor(
                    out=Fh[:, 1], in0=Gv[:, 0::2], in1=Gv[:, 1::2], op=sub
                )
                # W level: F (hb, hr16, wh128, wl2) -> G (hb, wb, hr16, wh128)
                Fw = F.rearrange(
                    "p (hb hr wh wl) -> p hb hr wh wl", hb=2, hr=16, wl=2
                )
                Gw = G.rearrange(
                    "p (hb wb hr wh) -> p hb wb hr wh", hb=2, wb=2, hr=16
                )
                V.tensor_tensor(
                    out=Gw[:, :, 0],
                    in0=Fw[:, :, :, :, 0], in1=Fw[:, :, :, :, 1], op=add,
                )
                WM.tensor_tensor(
                    out=Gw[:, :, 1],
                    in0=Fw[:, :, :, :, 0], in1=Fw[:, :, :, :, 1], op=sub,
                )
                if out_mode == "contig":
                    Gs = G.rearrange("p (hb wb rw) -> p hb wb rw", hb=2, wb=2)
                    nc.sync.dma_start(out=yv[c, 0], in_=Gs[0:64])
                    nc.scalar.dma_start(out=yv[c, 1], in_=Gs[64:128])
                else:
                    Gt = G.rearrange("(q two) f -> two q f", two=2)
                    for tb, ring in ((0, nc.sync), (1, nc.scalar)):
                        src_ = Gt[tb].rearrange(
                            "q (hb wb rw) -> q hb wb rw", hb=2, wb=2
                        )
                        ring.dma_start(out=yv[c, tb], in_=src_)

            if loop:
                with tc.For_i(0, reps):
                    xbs = [in_stage(c) for c in range(3)]
                    for c in range(3):
                        channel(c, xbs[c])
            else:
                for rep in range(reps):
                    xbs = [in_stage(c) for c in range(3)]
                    for c in range(3):
                        channel(c, xbs[c])

    nc.finalize()
    _NC_CACHE[key] = nc
    return nc


def _shard(x):
    x = np.ascontiguousarray(np.asarray(x, dtype=np.float32))
    assert x.shape == (4, 3, 32, 256, 256), x.shape
    in_maps = []
    for k in range(8):
        b, hh = divmod(k, 2)
        in_maps.append(
            {"x_shard": np.ascontiguousarray(x[b, :, :, 128 * hh : 128 * hh + 128])}
        )
    return in_maps


def _run(x, trace=False, **spmd_kwargs):
    from concourse.bass_utils import run_bass_kernel_spmd

    nc = _build()
    in_maps = _shard(x)
    bkr = run_bass_kernel_spmd(nc, in_maps, list(range(8)), trace=trace, **spmd_kwargs)

    out = np.empty((4, 24, 16, 128, 128), dtype=np.float32)
    for k in range(8):
        b, hh = divmod(k, 2)
        out[b, :, :, 64 * hh : 64 * hh + 64] = np.asarray(bkr.results[k]["y_shard"])
    return out, bkr


def kernel(x):
    out, _ = _run(x)
    return out
